# revision 20
# baseline (speedup 1.0000x reference)
"""Trainium2 Bass kernel for nn_Network_4655744548946 (plane-time hash-grid NeRF + MoE micro-MLPs).

Sharding: data-parallel over the 32768 points -> 8 cores x 4096 points.
Tables / kn_params replicated; gathers + grouped GEMMs local per shard.

Per-core layout: point pt = par*32 + j  (par in 0..127, j in 0..31).
"""

import numpy as np

L = 16
T = 1 << 19
D = 2
P = 128
NJ = 32
NPT = P * NJ           # 4096 points per core
NCORE = 8
NCH = 8                # MoE chunks of 512 points
CH = 512

RES = np.floor(16.0 * np.exp(np.arange(L) * np.log(64.0) / (L - 1))).astype(np.float32)
P2 = 2654435761
P3 = 805459861
P2M = P2 % T
P2LO = float(P2M % 1024)
P2HI = float(P2M // 1024)
MASK19 = T - 1
TWO_PI = 6.283185307179586
HALF_PI = 1.5707963267948966

_CACHE = {}


def _build():
    if 'nc' in _CACHE:
        return _CACHE['nc']
    from concourse import bass, bacc, mybir
    import concourse.tile as tile
    from concourse.masks import make_identity

    Op = mybir.AluOpType
    AF = mybir.ActivationFunctionType
    AX = mybir.AxisListType
    F32 = mybir.dt.float32
    I32 = mybir.dt.int32
    SHL = next(x for x in ('logical_shift_left', 'shift_left_logical', 'logical_shl',
                           'shl', 'lshift') if hasattr(Op, x))
    SHL = getattr(Op, SHL)

    nc = bacc.Bacc(num_swdge_queues=4)

    def dram(name, shape, dtype=F32, out=False):
        h = nc.declare_dram_parameter(name, list(shape), dtype, out)
        pat = []
        step = 1
        for s in reversed(shape):
            pat.append([step, s])
            step *= s
        return bass.AP(h, 0, list(reversed(pat)))

    xs = dram('xs', [P, 96])                 # x coords, [par, c*32+j]
    xsT = dram('xsT', [3, NPT])              # x transposed (routing)
    vs12 = dram('vs12', [12, NPT])           # viewdir rows (f*3+c)
    tabs = [dram(f'tab{i}', [L * T, D]) for i in range(3)]
    knr = dram('knr', [48 * 121, 32])        # repacked W1+b1 (lhsT-ready rows)
    knw2 = dram('knw2', [48 * 32, 3])        # repacked W2
    c_res = dram('c_res', [P, L])
    c_rm1 = dram('c_rm1', [P, L])
    c_ht = dram('c_ht', [P, 2 * L], I32)     # l*T + hash_t(k), col k*16+l
    c_wt = dram('c_wt', [P, 2 * L])          # t-weights, col k*16+l
    c_cg = dram('c_cg', [P, 1])              # par // 32
    c_fs = dram('c_fs', [12, 1])             # 2^f for row f*3+c
    c_A = dram('c_A', [3, 3])                # routing matrix (lhsT)
    c_one = dram('c_one', [1, NPT])          # ones row for netin[120]
    rgb = dram('rgb', [3, NPT], out=True)

    def ix(t, dim, i):
        sl = [slice(None)] * len(t.shape)
        sl[dim] = slice(i, i + 1)
        return t[tuple(sl)].squeeze(dim)

    def reAP(t, extra, dims):
        return bass.AP(t.tensor, t.offset + extra, [list(t.ap[0])] + [list(d) for d in dims])

    tc = tile.TileContext(nc)
    tc.__enter__()

    cp = tc.alloc_tile_pool(name='const', bufs=1)
    keep = tc.alloc_tile_pool(name='keep', bufs=1)
    scrp = tc.alloc_tile_pool(name='scr', bufs=1)
    gtp = tc.alloc_tile_pool(name='gt', bufs=1)
    psp = tc.alloc_tile_pool(name='ps', bufs=1, space='PSUM')

    def S(shape=(P, L, NJ), dtype=F32):
        return scrp.tile(list(shape), dtype, tag='s', bufs=7, name='s')

    # ---- constants into SBUF ----
    res_sb = cp.tile([P, L], F32)
    rm1_sb = cp.tile([P, L], F32)
    ht_sb = cp.tile([P, 2 * L], I32)
    wt_sb = cp.tile([P, 2 * L], F32)
    cg_sb = cp.tile([P, 1], F32)
    fs_sb = cp.tile([12, 1], F32)
    cA_sb = cp.tile([3, 3], F32)
    x_all = cp.tile([P, 96], F32)
    ident = cp.tile([P, P], F32)
    ones_sb = cp.tile([1, P], F32)
    for dst, src in ((res_sb, c_res), (rm1_sb, c_rm1), (ht_sb, c_ht), (wt_sb, c_wt),
                     (cg_sb, c_cg), (fs_sb, c_fs), (cA_sb, c_A), (x_all, xs)):
        nc.sync.dma_start(out=dst, in_=src)
    make_identity(nc, ident)
    nc.gpsimd.memset(ones_sb, 1.0)

    # ---- micro-MLP weights (12 groups of 4 nets) ----
    w1s, w2s = [], []
    for G in range(12):
        w1t = cp.tile([121, P], F32, tag='w1', bufs=12)
        nc.sync.dma_start(
            out=reAP(w1t, 0, [[32, 4], [1, 32]]),
            in_=bass.AP(knr.tensor, G * 4 * 121 * 32, [[32, 121], [121 * 32, 4], [1, 32]]))
        w2t = cp.tile([P, 3], F32, tag='w2', bufs=12)
        nc.sync.dma_start(out=w2t, in_=knw2[G * P:(G + 1) * P, :])
        w1s.append(w1t)
        w2s.append(w2t)

    # persistent big tensors
    netin = keep.tile([121, NPT], F32, tag='netin')
    NET = keep.tile([3, NPT], F32, tag='NET')
    nc.sync.dma_start(out=netin[120:121, :], in_=c_one)

    resb = res_sb.unsqueeze(2).broadcast_to((P, L, NJ))
    rm1b = rm1_sb.unsqueeze(2).broadcast_to((P, L, NJ))

    # ---- per-coordinate machinery ----
    fracs, has, hbs = {}, {}, {}
    for c in range(3):
        xb = x_all[:, c * NJ:(c + 1) * NJ].unsqueeze(1).broadcast_to((P, L, NJ))
        pos = S()
        nc.vector.tensor_tensor(out=pos, in0=xb, in1=resb, op=Op.mult)
        posm = S()
        nc.vector.tensor_tensor(out=posm, in0=pos, in1=rm1b, op=Op.min)
        fi = S(dtype=I32)
        nc.vector.tensor_copy(out=fi, in_=posm)
        ff = S()
        nc.vector.tensor_copy(out=ff, in_=fi)
        gt = S()
        nc.vector.tensor_tensor(out=gt, in0=ff, in1=posm, op=Op.is_gt)
        f0 = keep.tile([P, L, NJ], F32, tag='f0', bufs=1)
        nc.vector.tensor_tensor(out=f0, in0=ff, in1=gt, op=Op.subtract)
        fr = keep.tile([P, L, NJ], F32, tag=f'frac{c}')
        nc.vector.tensor_tensor(out=fr, in0=posm, in1=f0, op=Op.subtract)
        fracs[c] = fr
        if c in (0, 1):  # identity-prime corner ints (coord a)
            h0 = keep.tile([P, L, NJ], I32, tag=f'ha{c}0')
            nc.vector.tensor_copy(out=h0, in_=f0)
            h1 = keep.tile([P, L, NJ], I32, tag=f'ha{c}1')
            nc.vector.tensor_scalar(out=h1, in0=h0, scalar1=1.0, scalar2=None, op0=Op.add)
            has[c] = (h0, h1)
        if c in (1, 2):  # P2-hashed corner ints (coord b)
            mlo0 = S()
            nc.vector.tensor_scalar(out=mlo0, in0=f0, scalar1=P2LO, scalar2=None, op0=Op.mult)
            mlo1 = S()
            nc.vector.tensor_scalar(out=mlo1, in0=mlo0, scalar1=P2LO, scalar2=None, op0=Op.add)
            mhi0 = S()
            nc.vector.tensor_scalar(out=mhi0, in0=f0, scalar1=P2HI, scalar2=None, op0=Op.mult)
            mhi1 = S()
            nc.vector.tensor_scalar(out=mhi1, in0=mhi0, scalar1=P2HI, scalar2=None, op0=Op.add)
            outpair = []
            for msrc_lo, msrc_hi, kk in ((mlo0, mhi0, 0), (mlo1, mhi1, 1)):
                ilo = S(dtype=I32)
                nc.vector.tensor_copy(out=ilo, in_=msrc_lo)
                ihi = S(dtype=I32)
                nc.vector.tensor_copy(out=ihi, in_=msrc_hi)
                hi = S(dtype=I32)
                nc.vector.tensor_scalar(out=hi, in0=ihi, scalar1=511, scalar2=10,
                                        op0=Op.bitwise_and, op1=SHL)
                sm = S(dtype=I32)
                nc.vector.tensor_tensor(out=sm, in0=ilo, in1=hi, op=Op.add)
                hb = keep.tile([P, L, NJ], I32, tag=f'hb{c}{kk}')
                nc.vector.tensor_scalar(out=hb, in0=sm, scalar1=MASK19, scalar2=None,
                                        op0=Op.bitwise_and)
                outpair.append(hb)
            hbs[c] = tuple(outpair)

    # ---- planes: indices, weights, gathers, interp, transposes ----
    PLANES = ((0, 1), (0, 2), (1, 2))
    for p, (a, b) in enumerate(PLANES):
        ha, hb = has[a], hbs[b]
        fa, fb = fracs[a], fracs[b]
        OFF = keep.tile([P, L, 8, NJ], I32, tag='OFF', bufs=1)
        W = keep.tile([P, L, 8, NJ], F32, tag='W', bufs=2)
        q = {}
        for i in range(2):
            for jj in range(2):
                qt = S(dtype=I32)
                nc.vector.tensor_tensor(out=qt, in0=ha[i], in1=hb[jj], op=Op.bitwise_xor)
                q[(i, jj)] = qt
        for r in range(8):
            i, jj, k = r >> 2, (r >> 1) & 1, r & 1
            htb = ht_sb[:, k * L:(k + 1) * L].unsqueeze(2).broadcast_to((P, L, NJ))
            nc.vector.tensor_tensor(out=ix(OFF, 2, r), in0=q[(i, jj)], in1=htb,
                                    op=Op.bitwise_xor)
        wa0 = S()
        nc.vector.tensor_scalar(out=wa0, in0=fa, scalar1=-1.0, scalar2=1.0,
                                op0=Op.mult, op1=Op.add)
        wb0 = S()
        nc.vector.tensor_scalar(out=wb0, in0=fb, scalar1=-1.0, scalar2=1.0,
                                op0=Op.mult, op1=Op.add)
        wsel = {0: (wa0, wb0), 1: (fa, fb)}
        pab = {}
        for i in range(2):
            for jj in range(2):
                pt = S()
                nc.vector.tensor_tensor(out=pt, in0=wsel[i][0], in1=wsel[jj][1], op=Op.mult)
                pab[(i, jj)] = pt
        for r in range(8):
            i, jj, k = r >> 2, (r >> 1) & 1, r & 1
            wtb = wt_sb[:, k * L:(k + 1) * L].unsqueeze(2).broadcast_to((P, L, NJ))
            nc.vector.tensor_tensor(out=ix(W, 2, r), in0=pab[(i, jj)], in1=wtb, op=Op.mult)

        gts = []
        for l in range(L):
            g = gtp.tile([P, 8, NJ, D], F32, tag='g', bufs=3)
            gflat = reAP(g, 0, [[D, 8 * NJ], [1, D]])
            nc.gpsimd.indirect_dma_start(
                out=gflat, out_offset=None, in_=tabs[p],
                in_offset=bass.IndirectOffsetOnAxis(ap=ix(OFF, 1, l), axis=0))
            gts.append(g)

        FEAT = keep.tile([P, NJ, L, D], F32, tag=f'FEAT{p}')
        for l in range(L):
            m = S((P, 8, NJ, D))
            wbc = ix(W, 1, l).unsqueeze(3).broadcast_to((P, 8, NJ, D))
            nc.vector.tensor_tensor(out=m, in0=gts[l], in1=wbc, op=Op.mult)
            nc.vector.tensor_reduce(out=reAP(FEAT, l * D, [[L * D, NJ], [1, D]]),
                                    in_=reAP(m, 0, [[D, NJ], [1, D], [NJ * D, 8]]),
                                    axis=AX.X, op=Op.add)

        # netin columns are j-major: col = j*128 + i  (point n = i*32 + j)
        for jg in range(8):
            ptt = psp.tile([32, 4 * P], F32, tag='pt', bufs=2)
            for js in range(4):
                j = jg * 4 + js
                src = reAP(FEAT, j * L * D, [[1, L * D]])
                nc.tensor.transpose(ptt[:, js * P:(js + 1) * P], src, ident)
            pts = scrp.tile([32, 4 * P], F32, tag='pts', bufs=2)
            nc.scalar.activation(out=pts, in_=ptt, func=AF.Copy)
            nc.sync.dma_start(out=netin[p * 32:(p + 1) * 32, jg * 512:(jg + 1) * 512],
                              in_=pts)

    # ---- fourier rows 96..119 ----
    for n in range(NCH):
        sl = slice(n * CH, (n + 1) * CH)
        vL = scrp.tile([12, CH], F32, tag='vL', bufs=2)
        nc.sync.dma_start(out=vL, in_=vs12[:, sl])
        for sc in range(2):
            ang = scrp.tile([12, CH], F32, tag='f12', bufs=8)
            if sc == 0:
                nc.vector.tensor_scalar(out=ang, in0=vL, scalar1=fs_sb[:, 0:1],
                                        scalar2=None, op0=Op.mult)
            else:
                nc.vector.tensor_scalar(out=ang, in0=vL, scalar1=fs_sb[:, 0:1],
                                        scalar2=HALF_PI, op0=Op.mult, op1=Op.add)
            s = scrp.tile([12, CH], F32, tag='f12', bufs=8)
            nc.vector.tensor_scalar(out=s, in0=ang, scalar1=1.0 / TWO_PI, scalar2=0.5,
                                    op0=Op.mult, op1=Op.add)
            qi = scrp.tile([12, CH], I32, tag='f12', bufs=8)
            nc.vector.tensor_copy(out=qi, in_=s)
            qf = scrp.tile([12, CH], F32, tag='f12', bufs=8)
            nc.vector.tensor_copy(out=qf, in_=qi)
            gt = scrp.tile([12, CH], F32, tag='f12', bufs=8)
            nc.vector.tensor_tensor(out=gt, in0=qf, in1=s, op=Op.is_gt)
            q2 = scrp.tile([12, CH], F32, tag='f12', bufs=8)
            nc.vector.tensor_tensor(out=q2, in0=qf, in1=gt, op=Op.subtract)
            m1 = scrp.tile([12, CH], F32, tag='f12', bufs=8)
            nc.vector.tensor_scalar(out=m1, in0=q2, scalar1=-TWO_PI, scalar2=None,
                                    op0=Op.mult)
            red = scrp.tile([12, CH], F32, tag='f12', bufs=8)
            nc.vector.tensor_tensor(out=red, in0=m1, in1=ang, op=Op.add)
            fsin = scrp.tile([12, CH], F32, tag='fsin', bufs=2)
            nc.scalar.activation(out=fsin, in_=red, func=AF.Sin)
            nc.sync.dma_start(out=netin[96 + 12 * sc:108 + 12 * sc, sl], in_=fsin)

    # ---- routing net ids ----
    for n in range(NCH):
        sl = slice(n * CH, (n + 1) * CH)
        xL = scrp.tile([3, CH], F32, tag='xL', bufs=2)
        nc.sync.dma_start(out=xL, in_=xsT[:, sl])
        p4 = scrp.tile([3, CH], F32, tag='f3', bufs=5)
        nc.vector.tensor_scalar(out=p4, in0=xL, scalar1=4.0, scalar2=None, op0=Op.mult)
        qi = scrp.tile([3, CH], I32, tag='f3', bufs=5)
        nc.vector.tensor_copy(out=qi, in_=p4)
        qf = scrp.tile([3, CH], F32, tag='f3', bufs=5)
        nc.vector.tensor_copy(out=qf, in_=qi)
        gt = scrp.tile([3, CH], F32, tag='f3', bufs=5)
        nc.vector.tensor_tensor(out=gt, in0=qf, in1=p4, op=Op.is_gt)
        ij = scrp.tile([3, CH], F32, tag='f3', bufs=5)
        nc.vector.tensor_tensor(out=ij, in0=qf, in1=gt, op=Op.subtract)
        prt = psp.tile([3, CH], F32, tag='pr', bufs=3)
        nc.tensor.matmul(prt, cA_sb, ij, start=True, stop=True)
        nc.scalar.activation(out=NET[:, sl], in_=prt, func=AF.Copy)

    # ---- MoE: masked grouped GEMMs ----
    for n in range(NCH):
        sl = slice(n * CH, (n + 1) * CH)
        rgbp = psp.tile([3, CH], F32, tag='pr', bufs=3)
        acc = 0
        for p in range(3):
            nrow = scrp.tile([1, CH], F32, tag='nrow', bufs=2)
            nc.sync.dma_start(out=nrow, in_=NET[p:p + 1, sl])
            netbp = psp.tile([P, CH], F32, tag='pt', bufs=2)
            nc.tensor.matmul(netbp, ones_sb, nrow, start=True, stop=True)
            for g in range(4):
                G = p * 4 + g
                mask = scrp.tile([P, CH], F32, tag='s', bufs=7)
                nc.vector.tensor_scalar(out=mask, in0=netbp, scalar1=cg_sb[:, 0:1],
                                        scalar2=float(4 * g), op0=Op.subtract,
                                        op1=Op.is_equal)
                h1p = psp.tile([P, CH], F32, tag='ph', bufs=3)
                nc.tensor.matmul(h1p, w1s[G], netin[0:121, sl], start=True, stop=True)
                h1s = scrp.tile([P, CH], F32, tag='s', bufs=7)
                nc.scalar.activation(out=h1s, in_=h1p, func=AF.Relu)
                h1m = scrp.tile([P, CH], F32, tag='s', bufs=7)
                nc.vector.tensor_tensor(out=h1m, in0=h1s, in1=mask, op=Op.mult)
                nc.tensor.matmul(rgbp, w2s[G], h1m, start=(acc == 0), stop=(acc == 11))
                acc += 1
        osb = scrp.tile([3, CH], F32, tag='osb', bufs=2)
        nc.scalar.activation(out=osb, in_=rgbp, func=AF.Copy, scale=1.0 / 3.0)
        nc.sync.dma_start(out=rgb[:, sl], in_=osb)

    for pool in (psp, gtp, scrp, keep, cp):
        pool.release()
    tc.__exit__(None, None, None)
    nc.finalize()
    _CACHE['nc'] = nc
    return nc


def _host_prep(norm, viewdir, t, table_xyt, table_xzt, table_yzt, kn_params):
    x = np.ascontiguousarray(norm.reshape(NCORE * NPT, 3), dtype=np.float32)
    v = np.ascontiguousarray(viewdir.reshape(NCORE * NPT, 3), dtype=np.float32)
    tabs = [np.ascontiguousarray(tt.reshape(L * T, D), dtype=np.float32)
            for tt in (table_xyt, table_xzt, table_yzt)]
    kn = np.asarray(kn_params, dtype=np.float32)

    W1 = kn[:, :3840].reshape(48, 120, 32)
    b1 = kn[:, 3840:3872].reshape(48, 1, 32)
    perm = np.array([96 + c * 8 + sc * 4 + f
                     for sc in range(2) for f in range(4) for c in range(3)])
    knr = np.concatenate([W1[:, :96], W1[:, perm], b1], axis=1).reshape(48 * 121, 32)
    knr = np.ascontiguousarray(knr)
    knw2 = np.ascontiguousarray(kn[:, 3872:].reshape(48 * 32, 3))

    tt0 = np.float32(t.reshape(-1)[0])
    pos_t = np.clip(tt0 * RES, np.float32(0.0), RES - np.float32(1.0)).astype(np.float32)
    f_t = np.floor(pos_t).astype(np.float32)
    frac_t = (pos_t - f_t).astype(np.float32)
    ct = (f_t[None, :] + np.arange(2, dtype=np.float32)[:, None]).astype(np.uint32)
    ht = (ct * np.uint32(P3)) % np.uint32(T)
    ht_full = (ht.astype(np.int64) + (np.arange(L, dtype=np.int64) * T)[None, :]).astype(np.int32)
    wt = np.stack([np.float32(1.0) - frac_t, frac_t]).astype(np.float32)  # [2, L]

    consts = {
        'c_res': np.tile(RES, (P, 1)).astype(np.float32),
        'c_rm1': np.tile(RES - np.float32(1.0), (P, 1)).astype(np.float32),
        'c_ht': np.tile(ht_full.reshape(1, 2 * L), (P, 1)).astype(np.int32),
        'c_wt': np.tile(wt.reshape(1, 2 * L), (P, 1)).astype(np.float32),
        'c_cg': (np.arange(P, dtype=np.float32) // 32).reshape(P, 1).astype(np.float32),
        'c_fs': (2.0 ** (np.arange(12) // 3)).astype(np.float32).reshape(12, 1),
        'c_A': np.array([[4, 4, 0], [1, 0, 4], [0, 1, 1]], dtype=np.float32),
        'c_one': np.ones((1, NPT), dtype=np.float32),
    }

    in_maps = []
    for core in range(NCORE):
        sl = slice(core * NPT, (core + 1) * NPT)
        xc = x[sl]
        # device column order is j-major: col = j*128 + i for point n = i*32 + j
        xc_col = xc.reshape(P, NJ, 3).transpose(1, 0, 2).reshape(NPT, 3)
        vc_col = v[sl].reshape(P, NJ, 3).transpose(1, 0, 2).reshape(NPT, 3)
        m = {
            'xs': np.ascontiguousarray(
                xc.reshape(P, NJ, 3).transpose(0, 2, 1).reshape(P, 96)),
            'xsT': np.ascontiguousarray(xc_col.T),
            'vs12': np.ascontiguousarray(np.tile(vc_col.T, (4, 1))),
            'tab0': tabs[0], 'tab1': tabs[1], 'tab2': tabs[2],
            'knr': knr, 'knw2': knw2,
        }
        m.update(consts)
        in_maps.append(m)
    return in_maps


def kernel(norm, viewdir, t, table_xyt, table_xzt, table_yzt, kn_params):
    from concourse.bass_utils import run_bass_kernel_spmd
    nc = _build()
    in_maps = _host_prep(norm, viewdir, t, table_xyt, table_xzt, table_yzt, kn_params)
    res = run_bass_kernel_spmd(nc, in_maps, core_ids=list(range(NCORE)))
    outs = res.results
    full = np.concatenate(
        [np.asarray(outs[c]['rgb']).reshape(3, NJ, P).transpose(2, 1, 0).reshape(NPT, 3)
         for c in range(NCORE)], axis=0)
    return full.reshape(1, NCORE * NPT, 3).astype(np.float32)


# revision 26
# speedup vs baseline: 1.0325x; 1.0325x over previous
"""Trainium2 Bass kernel for nn_Network_4655744548946 (plane-time hash-grid NeRF + MoE micro-MLPs).

Sharding: data-parallel over the 32768 points -> 8 cores x 4096 points.
Tables / kn_params replicated; gathers + grouped GEMMs local per shard.

Per-core layout: point pt = par*32 + j  (par in 0..127, j in 0..31).
"""

import numpy as np

L = 16
T = 1 << 19
D = 2
P = 128
NJ = 32
NPT = P * NJ           # 4096 points per core
NCORE = 8
NCH = 8                # MoE chunks of 512 points
CH = 512

RES = np.floor(16.0 * np.exp(np.arange(L) * np.log(64.0) / (L - 1))).astype(np.float32)
P2 = 2654435761
P3 = 805459861
P2M = P2 % T
P2LO = float(P2M % 1024)
P2HI = float(P2M // 1024)
MASK19 = T - 1
TWO_PI = 6.283185307179586
HALF_PI = 1.5707963267948966

_CACHE = {}


def _build():
    if 'nc' in _CACHE:
        return _CACHE['nc']
    from concourse import bass, bacc, mybir
    import concourse.tile as tile
    from concourse.masks import make_identity

    Op = mybir.AluOpType
    AF = mybir.ActivationFunctionType
    AX = mybir.AxisListType
    F32 = mybir.dt.float32
    I32 = mybir.dt.int32
    SHL = next(x for x in ('logical_shift_left', 'shift_left_logical', 'logical_shl',
                           'shl', 'lshift') if hasattr(Op, x))
    SHL = getattr(Op, SHL)

    nc = bacc.Bacc(num_swdge_queues=4)

    def dram(name, shape, dtype=F32, out=False):
        h = nc.declare_dram_parameter(name, list(shape), dtype, out)
        pat = []
        step = 1
        for s in reversed(shape):
            pat.append([step, s])
            step *= s
        return bass.AP(h, 0, list(reversed(pat)))

    xs = dram('xs', [P, 96])                 # x coords, [par, c*32+j]
    xsT = dram('xsT', [3, NPT])              # x transposed (routing)
    vs12 = dram('vs12', [12, NPT])           # viewdir rows (f*3+c)
    tabs = [dram(f'tab{i}', [L * T, D]) for i in range(3)]
    knr = dram('knr', [48 * 121, 32])        # repacked W1+b1 (lhsT-ready rows)
    knw2 = dram('knw2', [48 * 32, 3])        # repacked W2
    c_res = dram('c_res', [P, L])
    c_rm1 = dram('c_rm1', [P, L])
    c_ht = dram('c_ht', [P, 2 * L], I32)     # l*T + hash_t(k), col k*16+l
    c_wt = dram('c_wt', [P, 2 * L])          # t-weights, col k*16+l
    c_cg = dram('c_cg', [P, 1])              # par // 32
    c_fs = dram('c_fs', [12, 1])             # 2^f for row f*3+c
    c_A = dram('c_A', [3, 3])                # routing matrix (lhsT)
    c_one = dram('c_one', [1, NPT])          # ones row for netin[120]
    rgb = dram('rgb', [3, NPT], out=True)

    def ix(t, dim, i):
        sl = [slice(None)] * len(t.shape)
        sl[dim] = slice(i, i + 1)
        return t[tuple(sl)].squeeze(dim)

    def reAP(t, extra, dims):
        return bass.AP(t.tensor, t.offset + extra, [list(t.ap[0])] + [list(d) for d in dims])

    tc = tile.TileContext(nc)
    tc.__enter__()

    cp = tc.alloc_tile_pool(name='const', bufs=1)
    keep = tc.alloc_tile_pool(name='keep', bufs=1)
    scrp = tc.alloc_tile_pool(name='scr', bufs=1)
    gtp = tc.alloc_tile_pool(name='gt', bufs=1)
    psp = tc.alloc_tile_pool(name='ps', bufs=1, space='PSUM')

    def S(shape=(P, L, NJ), dtype=F32):
        return scrp.tile(list(shape), dtype, tag='s', bufs=7, name='s')

    # ---- constants into SBUF ----
    res_sb = cp.tile([P, L], F32)
    rm1_sb = cp.tile([P, L], F32)
    ht_sb = cp.tile([P, 2 * L], I32)
    wt_sb = cp.tile([P, 2 * L], F32)
    cg_sb = cp.tile([P, 1], F32)
    fs_sb = cp.tile([12, 1], F32)
    cA_sb = cp.tile([3, 3], F32)
    x_all = cp.tile([P, 96], F32)
    ident = cp.tile([P, P], F32)
    ones_sb = cp.tile([1, P], F32)
    for dst, src in ((res_sb, c_res), (rm1_sb, c_rm1), (ht_sb, c_ht), (wt_sb, c_wt),
                     (cg_sb, c_cg), (fs_sb, c_fs), (cA_sb, c_A), (x_all, xs)):
        nc.sync.dma_start(out=dst, in_=src)
    make_identity(nc, ident)
    nc.gpsimd.memset(ones_sb, 1.0)

    # ---- micro-MLP weights (12 groups of 4 nets) ----
    w1s, w2s = [], []
    for G in range(12):
        w1t = cp.tile([121, P], F32, tag='w1', bufs=12)
        nc.sync.dma_start(
            out=reAP(w1t, 0, [[32, 4], [1, 32]]),
            in_=bass.AP(knr.tensor, G * 4 * 121 * 32, [[32, 121], [121 * 32, 4], [1, 32]]))
        w2t = cp.tile([P, 3], F32, tag='w2', bufs=12)
        nc.sync.dma_start(out=w2t, in_=knw2[G * P:(G + 1) * P, :])
        w1s.append(w1t)
        w2s.append(w2t)
    BF16 = mybir.dt.bfloat16
    w1h, w1l, w2h, w2l = [], [], [], []
    for G in range(12):
        a = cp.tile([121, P], BF16, tag='w1h', bufs=12)
        nc.vector.tensor_copy(out=a, in_=w1s[G])
        b = cp.tile([121, P], BF16, tag='w1l', bufs=12)
        nc.vector.tensor_tensor(out=b, in0=w1s[G], in1=a, op=Op.subtract)
        c2 = cp.tile([P, 3], BF16, tag='w2h', bufs=12)
        nc.vector.tensor_copy(out=c2, in_=w2s[G])
        d2 = cp.tile([P, 3], BF16, tag='w2l', bufs=12)
        nc.vector.tensor_tensor(out=d2, in0=w2s[G], in1=c2, op=Op.subtract)
        w1h.append(a); w1l.append(b); w2h.append(c2); w2l.append(d2)

    # persistent big tensors
    netin = keep.tile([121, NPT], F32, tag='netin')
    NET = keep.tile([3, NPT], F32, tag='NET')
    nc.sync.dma_start(out=netin[120:121, :], in_=c_one)

    resb = res_sb.unsqueeze(2).broadcast_to((P, L, NJ))
    rm1b = rm1_sb.unsqueeze(2).broadcast_to((P, L, NJ))

    # ---- per-coordinate machinery ----
    fracs, has, hbs = {}, {}, {}
    for c in range(3):
        xb = x_all[:, c * NJ:(c + 1) * NJ].unsqueeze(1).broadcast_to((P, L, NJ))
        pos = S()
        nc.vector.tensor_tensor(out=pos, in0=xb, in1=resb, op=Op.mult)
        posm = S()
        nc.vector.tensor_tensor(out=posm, in0=pos, in1=rm1b, op=Op.min)
        fi = S(dtype=I32)
        nc.vector.tensor_copy(out=fi, in_=posm)
        ff = S()
        nc.vector.tensor_copy(out=ff, in_=fi)
        gt = S()
        nc.vector.tensor_tensor(out=gt, in0=ff, in1=posm, op=Op.is_gt)
        f0 = keep.tile([P, L, NJ], F32, tag='f0', bufs=1)
        nc.vector.tensor_tensor(out=f0, in0=ff, in1=gt, op=Op.subtract)
        fr = keep.tile([P, L, NJ], F32, tag=f'frac{c}')
        nc.vector.tensor_tensor(out=fr, in0=posm, in1=f0, op=Op.subtract)
        fracs[c] = fr
        if c in (0, 1):  # identity-prime corner ints (coord a)
            h0 = keep.tile([P, L, NJ], I32, tag=f'ha{c}0')
            nc.vector.tensor_copy(out=h0, in_=f0)
            h1 = keep.tile([P, L, NJ], I32, tag=f'ha{c}1')
            nc.vector.tensor_scalar(out=h1, in0=h0, scalar1=1.0, scalar2=None, op0=Op.add)
            has[c] = (h0, h1)
        if c in (1, 2):  # P2-hashed corner ints (coord b)
            mlo0 = S()
            nc.vector.tensor_scalar(out=mlo0, in0=f0, scalar1=P2LO, scalar2=None, op0=Op.mult)
            mlo1 = S()
            nc.vector.tensor_scalar(out=mlo1, in0=mlo0, scalar1=P2LO, scalar2=None, op0=Op.add)
            mhi0 = S()
            nc.vector.tensor_scalar(out=mhi0, in0=f0, scalar1=P2HI, scalar2=None, op0=Op.mult)
            mhi1 = S()
            nc.vector.tensor_scalar(out=mhi1, in0=mhi0, scalar1=P2HI, scalar2=None, op0=Op.add)
            outpair = []
            for msrc_lo, msrc_hi, kk in ((mlo0, mhi0, 0), (mlo1, mhi1, 1)):
                ilo = S(dtype=I32)
                nc.vector.tensor_copy(out=ilo, in_=msrc_lo)
                ihi = S(dtype=I32)
                nc.vector.tensor_copy(out=ihi, in_=msrc_hi)
                hi = S(dtype=I32)
                nc.vector.tensor_scalar(out=hi, in0=ihi, scalar1=511, scalar2=10,
                                        op0=Op.bitwise_and, op1=SHL)
                sm = S(dtype=I32)
                nc.vector.tensor_tensor(out=sm, in0=ilo, in1=hi, op=Op.add)
                hb = keep.tile([P, L, NJ], I32, tag=f'hb{c}{kk}')
                nc.vector.tensor_scalar(out=hb, in0=sm, scalar1=MASK19, scalar2=None,
                                        op0=Op.bitwise_and)
                outpair.append(hb)
            hbs[c] = tuple(outpair)

    # ---- planes: indices, weights, gathers, interp, transposes ----
    PLANES = ((0, 1), (0, 2), (1, 2))
    for p, (a, b) in enumerate(PLANES):
        ha, hb = has[a], hbs[b]
        fa, fb = fracs[a], fracs[b]
        OFF = keep.tile([P, L, 8, NJ], I32, tag='OFF', bufs=1)
        W = keep.tile([P, L, 8, NJ], F32, tag='W', bufs=2)
        q = {}
        for i in range(2):
            for jj in range(2):
                qt = S(dtype=I32)
                nc.vector.tensor_tensor(out=qt, in0=ha[i], in1=hb[jj], op=Op.bitwise_xor)
                q[(i, jj)] = qt
        for r in range(8):
            i, jj, k = r >> 2, (r >> 1) & 1, r & 1
            htb = ht_sb[:, k * L:(k + 1) * L].unsqueeze(2).broadcast_to((P, L, NJ))
            nc.vector.tensor_tensor(out=ix(OFF, 2, r), in0=q[(i, jj)], in1=htb,
                                    op=Op.bitwise_xor)
        wa0 = S()
        nc.vector.tensor_scalar(out=wa0, in0=fa, scalar1=-1.0, scalar2=1.0,
                                op0=Op.mult, op1=Op.add)
        wb0 = S()
        nc.vector.tensor_scalar(out=wb0, in0=fb, scalar1=-1.0, scalar2=1.0,
                                op0=Op.mult, op1=Op.add)
        wsel = {0: (wa0, wb0), 1: (fa, fb)}
        pab = {}
        for i in range(2):
            for jj in range(2):
                pt = S()
                nc.vector.tensor_tensor(out=pt, in0=wsel[i][0], in1=wsel[jj][1], op=Op.mult)
                pab[(i, jj)] = pt
        for r in range(8):
            i, jj, k = r >> 2, (r >> 1) & 1, r & 1
            wtb = wt_sb[:, k * L:(k + 1) * L].unsqueeze(2).broadcast_to((P, L, NJ))
            nc.vector.tensor_tensor(out=ix(W, 2, r), in0=pab[(i, jj)], in1=wtb, op=Op.mult)

        gts = []
        for l in range(L):
            g = gtp.tile([P, 8, NJ, D], F32, tag='g', bufs=3)
            gflat = reAP(g, 0, [[D, 8 * NJ], [1, D]])
            nc.gpsimd.indirect_dma_start(
                out=gflat, out_offset=None, in_=tabs[p],
                in_offset=bass.IndirectOffsetOnAxis(ap=ix(OFF, 1, l), axis=0))
            gts.append(g)

        FEAT = keep.tile([P, NJ, L, D], F32, tag=f'FEAT{p}')
        for l in range(L):
            m = S((P, 8, NJ, D))
            wbc = ix(W, 1, l).unsqueeze(3).broadcast_to((P, 8, NJ, D))
            nc.vector.tensor_tensor(out=m, in0=gts[l], in1=wbc, op=Op.mult)
            nc.vector.tensor_reduce(out=reAP(FEAT, l * D, [[L * D, NJ], [1, D]]),
                                    in_=reAP(m, 0, [[D, NJ], [1, D], [NJ * D, 8]]),
                                    axis=AX.X, op=Op.add)

        # netin columns are j-major: col = j*128 + i  (point n = i*32 + j)
        for jg in range(8):
            ptt = psp.tile([32, 4 * P], F32, tag='pt', bufs=2)
            for js in range(4):
                j = jg * 4 + js
                src = reAP(FEAT, j * L * D, [[1, L * D]])
                nc.tensor.transpose(ptt[:, js * P:(js + 1) * P], src, ident)
            pts = scrp.tile([32, 4 * P], F32, tag='pts', bufs=2)
            nc.scalar.activation(out=pts, in_=ptt, func=AF.Copy)
            nc.sync.dma_start(out=netin[p * 32:(p + 1) * 32, jg * 512:(jg + 1) * 512],
                              in_=pts)

    # ---- fourier rows 96..119 ----
    for n in range(NCH):
        sl = slice(n * CH, (n + 1) * CH)
        vL = scrp.tile([12, CH], F32, tag='vL', bufs=2)
        nc.sync.dma_start(out=vL, in_=vs12[:, sl])
        for sc in range(2):
            ang = scrp.tile([12, CH], F32, tag='f12', bufs=8)
            if sc == 0:
                nc.vector.tensor_scalar(out=ang, in0=vL, scalar1=fs_sb[:, 0:1],
                                        scalar2=None, op0=Op.mult)
            else:
                nc.vector.tensor_scalar(out=ang, in0=vL, scalar1=fs_sb[:, 0:1],
                                        scalar2=HALF_PI, op0=Op.mult, op1=Op.add)
            s = scrp.tile([12, CH], F32, tag='f12', bufs=8)
            nc.vector.tensor_scalar(out=s, in0=ang, scalar1=1.0 / TWO_PI, scalar2=0.5,
                                    op0=Op.mult, op1=Op.add)
            qi = scrp.tile([12, CH], I32, tag='f12', bufs=8)
            nc.vector.tensor_copy(out=qi, in_=s)
            qf = scrp.tile([12, CH], F32, tag='f12', bufs=8)
            nc.vector.tensor_copy(out=qf, in_=qi)
            gt = scrp.tile([12, CH], F32, tag='f12', bufs=8)
            nc.vector.tensor_tensor(out=gt, in0=qf, in1=s, op=Op.is_gt)
            q2 = scrp.tile([12, CH], F32, tag='f12', bufs=8)
            nc.vector.tensor_tensor(out=q2, in0=qf, in1=gt, op=Op.subtract)
            m1 = scrp.tile([12, CH], F32, tag='f12', bufs=8)
            nc.vector.tensor_scalar(out=m1, in0=q2, scalar1=-TWO_PI, scalar2=None,
                                    op0=Op.mult)
            red = scrp.tile([12, CH], F32, tag='f12', bufs=8)
            nc.vector.tensor_tensor(out=red, in0=m1, in1=ang, op=Op.add)
            fsin = scrp.tile([12, CH], F32, tag='fsin', bufs=2)
            nc.scalar.activation(out=fsin, in_=red, func=AF.Sin)
            nc.sync.dma_start(out=netin[96 + 12 * sc:108 + 12 * sc, sl], in_=fsin)

    # ---- routing net ids ----
    for n in range(NCH):
        sl = slice(n * CH, (n + 1) * CH)
        xL = scrp.tile([3, CH], F32, tag='xL', bufs=2)
        nc.sync.dma_start(out=xL, in_=xsT[:, sl])
        p4 = scrp.tile([3, CH], F32, tag='f3', bufs=5)
        nc.vector.tensor_scalar(out=p4, in0=xL, scalar1=4.0, scalar2=None, op0=Op.mult)
        qi = scrp.tile([3, CH], I32, tag='f3', bufs=5)
        nc.vector.tensor_copy(out=qi, in_=p4)
        qf = scrp.tile([3, CH], F32, tag='f3', bufs=5)
        nc.vector.tensor_copy(out=qf, in_=qi)
        gt = scrp.tile([3, CH], F32, tag='f3', bufs=5)
        nc.vector.tensor_tensor(out=gt, in0=qf, in1=p4, op=Op.is_gt)
        ij = scrp.tile([3, CH], F32, tag='f3', bufs=5)
        nc.vector.tensor_tensor(out=ij, in0=qf, in1=gt, op=Op.subtract)
        prt = psp.tile([3, CH], F32, tag='pr', bufs=3)
        nc.tensor.matmul(prt, cA_sb, ij, start=True, stop=True)
        nc.scalar.activation(out=NET[:, sl], in_=prt, func=AF.Copy)

    # ---- MoE: masked grouped GEMMs ----
    for n in range(NCH):
        sl = slice(n * CH, (n + 1) * CH)
        nh = scrp.tile([121, CH], BF16, tag='nh', bufs=2)
        nc.vector.tensor_copy(out=nh, in_=netin[0:121, sl])
        nl = scrp.tile([121, CH], BF16, tag='nl', bufs=2)
        nc.vector.tensor_tensor(out=nl, in0=netin[0:121, sl], in1=nh, op=Op.subtract)
        rgbp = psp.tile([3, CH], F32, tag='pr', bufs=3)
        acc = 0
        for p in range(3):
            nrow = scrp.tile([1, CH], F32, tag='nrow', bufs=2)
            nc.sync.dma_start(out=nrow, in_=NET[p:p + 1, sl])
            netbp = psp.tile([P, CH], F32, tag='pt', bufs=2)
            nc.tensor.matmul(netbp, ones_sb, nrow, start=True, stop=True)
            for g in range(4):
                G = p * 4 + g
                mask = scrp.tile([P, CH], F32, tag='s', bufs=7)
                nc.vector.tensor_scalar(out=mask, in0=netbp, scalar1=cg_sb[:, 0:1],
                                        scalar2=float(4 * g), op0=Op.subtract,
                                        op1=Op.is_equal)
                h1p = psp.tile([P, CH], F32, tag='ph', bufs=3)
                nc.tensor.matmul(h1p, w1h[G], nh, start=True, stop=False)
                nc.tensor.matmul(h1p, w1l[G], nh, start=False, stop=False)
                nc.tensor.matmul(h1p, w1h[G], nl, start=False, stop=True)
                h1s = scrp.tile([P, CH], F32, tag='s', bufs=7)
                nc.scalar.activation(out=h1s, in_=h1p, func=AF.Relu)
                h1m = scrp.tile([P, CH], F32, tag='s', bufs=7)
                nc.vector.tensor_tensor(out=h1m, in0=h1s, in1=mask, op=Op.mult)
                h1bh = scrp.tile([P, CH], BF16, tag='s2', bufs=2)
                nc.vector.tensor_copy(out=h1bh, in_=h1m)
                h1bl = scrp.tile([P, CH], BF16, tag='s2', bufs=2)
                nc.vector.tensor_tensor(out=h1bl, in0=h1m, in1=h1bh, op=Op.subtract)
                nc.tensor.matmul(rgbp, w2h[G], h1bh, start=(acc == 0), stop=False)
                nc.tensor.matmul(rgbp, w2l[G], h1bh, start=False, stop=False)
                nc.tensor.matmul(rgbp, w2h[G], h1bl, start=False, stop=(acc == 11))
                acc += 1
        osb = scrp.tile([3, CH], F32, tag='osb', bufs=2)
        nc.scalar.activation(out=osb, in_=rgbp, func=AF.Copy, scale=1.0 / 3.0)
        nc.sync.dma_start(out=rgb[:, sl], in_=osb)

    for pool in (psp, gtp, scrp, keep, cp):
        pool.release()
    tc.__exit__(None, None, None)
    nc.finalize()
    _CACHE['nc'] = nc
    return nc


def _host_prep(norm, viewdir, t, table_xyt, table_xzt, table_yzt, kn_params):
    x = np.ascontiguousarray(norm.reshape(NCORE * NPT, 3), dtype=np.float32)
    v = np.ascontiguousarray(viewdir.reshape(NCORE * NPT, 3), dtype=np.float32)
    tabs = [np.ascontiguousarray(tt.reshape(L * T, D), dtype=np.float32)
            for tt in (table_xyt, table_xzt, table_yzt)]
    kn = np.asarray(kn_params, dtype=np.float32)

    W1 = kn[:, :3840].reshape(48, 120, 32)
    b1 = kn[:, 3840:3872].reshape(48, 1, 32)
    perm = np.array([96 + c * 8 + sc * 4 + f
                     for sc in range(2) for f in range(4) for c in range(3)])
    knr = np.concatenate([W1[:, :96], W1[:, perm], b1], axis=1).reshape(48 * 121, 32)
    knr = np.ascontiguousarray(knr)
    knw2 = np.ascontiguousarray(kn[:, 3872:].reshape(48 * 32, 3))

    tt0 = np.float32(t.reshape(-1)[0])
    pos_t = np.clip(tt0 * RES, np.float32(0.0), RES - np.float32(1.0)).astype(np.float32)
    f_t = np.floor(pos_t).astype(np.float32)
    frac_t = (pos_t - f_t).astype(np.float32)
    ct = (f_t[None, :] + np.arange(2, dtype=np.float32)[:, None]).astype(np.uint32)
    ht = (ct * np.uint32(P3)) % np.uint32(T)
    ht_full = (ht.astype(np.int64) + (np.arange(L, dtype=np.int64) * T)[None, :]).astype(np.int32)
    wt = np.stack([np.float32(1.0) - frac_t, frac_t]).astype(np.float32)  # [2, L]

    consts = {
        'c_res': np.tile(RES, (P, 1)).astype(np.float32),
        'c_rm1': np.tile(RES - np.float32(1.0), (P, 1)).astype(np.float32),
        'c_ht': np.tile(ht_full.reshape(1, 2 * L), (P, 1)).astype(np.int32),
        'c_wt': np.tile(wt.reshape(1, 2 * L), (P, 1)).astype(np.float32),
        'c_cg': (np.arange(P, dtype=np.float32) // 32).reshape(P, 1).astype(np.float32),
        'c_fs': (2.0 ** (np.arange(12) // 3)).astype(np.float32).reshape(12, 1),
        'c_A': np.array([[4, 4, 0], [1, 0, 4], [0, 1, 1]], dtype=np.float32),
        'c_one': np.ones((1, NPT), dtype=np.float32),
    }

    in_maps = []
    for core in range(NCORE):
        sl = slice(core * NPT, (core + 1) * NPT)
        xc = x[sl]
        # device column order is j-major: col = j*128 + i for point n = i*32 + j
        xc_col = xc.reshape(P, NJ, 3).transpose(1, 0, 2).reshape(NPT, 3)
        vc_col = v[sl].reshape(P, NJ, 3).transpose(1, 0, 2).reshape(NPT, 3)
        m = {
            'xs': np.ascontiguousarray(
                xc.reshape(P, NJ, 3).transpose(0, 2, 1).reshape(P, 96)),
            'xsT': np.ascontiguousarray(xc_col.T),
            'vs12': np.ascontiguousarray(np.tile(vc_col.T, (4, 1))),
            'tab0': tabs[0], 'tab1': tabs[1], 'tab2': tabs[2],
            'knr': knr, 'knw2': knw2,
        }
        m.update(consts)
        in_maps.append(m)
    return in_maps


def kernel(norm, viewdir, t, table_xyt, table_xzt, table_yzt, kn_params):
    from concourse.bass_utils import run_bass_kernel_spmd
    nc = _build()
    in_maps = _host_prep(norm, viewdir, t, table_xyt, table_xzt, table_yzt, kn_params)
    res = run_bass_kernel_spmd(nc, in_maps, core_ids=list(range(NCORE)))
    outs = res.results
    full = np.concatenate(
        [np.asarray(outs[c]['rgb']).reshape(3, NJ, P).transpose(2, 1, 0).reshape(NPT, 3)
         for c in range(NCORE)], axis=0)
    return full.reshape(1, NCORE * NPT, 3).astype(np.float32)
